# revision 1
# baseline (speedup 1.0000x reference)
"""CASSI layer kernel for Trainium2 (8 NeuronCores, Bass/Tile).

Math (matches the reference nn_CASSI_layer):
    H2[m,n,s]        = H[0,m,n,0,s]
    Y[b,m,n+l,s]    += H2[m,n,s] * x[b,m,n,l]            (shear-sum, l in [0,24))
    sigm             = sum(Y^2) / (M*W*B*10^(40/10))
    Yn               = Y + sqrt(sigm) * noise_eps         (noise_eps broadcast over s)
    X[b,m,n,l]       = sum_s H2[m,n,s] * Yn[b,m,n+l,s]
    out              = X / max(X)

Distribution: the (b, m) pairs form 4*256 = 1024 independent rows; each of the
8 cores takes 128 rows (core c: b = c//2, m in [128*(c%2), 128*(c%2)+128)),
mapped onto the 128 SBUF partitions.  Everything per-row lives along the free
dimension, so the spectral shifts are plain address offsets (always 4-byte
aligned in fp16 because the shift stride is S=22 elements).

The two global scalar couplings (sigm, max) are linearized out of the device
kernel: X = X0 + sqrt(sigm)*Xn with X0 the noise-free result (device) and
Xn[b,m,n,l] = (sum_s H2[m,n,s]) * noise_eps[b,m,n+l] (cheap host outer
product).  The device returns X0 and per-partition sum(Y^2); the host applies
sigma, the noise term, and the global max normalization.

Engine split per core: ScalarE materializes the x-column broadcasts over the
s axis; VectorE runs fp16 multiplies/adds in the packed 2x perf mode (the
shear offsets l*S*2 bytes are all 4-byte aligned, and stage-4 pipelines are
pair-batched over l to amortize per-op overhead); GPSIMD owns independent
pipelines for the last few l values in both stages (a second Y accumulator in
stage 2, full mul+fold chains in stage 4); the s-contraction is a
22->16->8->4->2->1 aligned fold tree; and sum(Y^2) rides the ScalarE Square
activation's accumulator.  GPSIMD multiplies read the step-0 broadcast APs
directly (it has no packed perf modes to forfeit), so its chains start right
after the input DMAs with no ScalarE dependency; the first VectorE multiply
likewise runs 1x off the broadcast to skip the ScalarE ramp.  Engine
assignments were tuned with the calibrated instruction-cost timeline
simulator (316us all-VectorE -> 250.6us final; deeper GPSIMD assignment,
cross-engine fold handoffs, emission reorders, strided DMA prefetch/split,
and quad-chunking all measured worse, leaving VectorE's minimal stream --
stage-2 muls+adds, accumulator merge, stage-4 muls+fold trees -- as the
critical path, balanced within ~10us of the GPSIMD chains).
"""

from contextlib import ExitStack

import numpy as np

import concourse.bass as bass
import concourse.bacc as bacc
import concourse.tile as tile
from concourse import mybir
from concourse.bass_utils import run_bass_kernel_spmd

B, M, L, S = 4, 256, 24, 22
W = M + L - 1  # 279
N_CORES = 8
ROWS = 128  # (b, m) rows per core
NOISE_DB = 40.0

_F32 = mybir.dt.float32
_F16 = mybir.dt.float16


def build_bass(dtype=_F16, gps_s2=0, gps_hand_s4=0, tmp_bufs=2, rep_bufs=2, gps_indep_s4=5, gps_indep_s2=5, i2_bufs=2, g2tmp_bufs=1, gpool_bufs=2, s2_stride=100, s4_chunk=4, gps_merge=False, dma_s2=False, s2_chunk=0, s2_chunk_from=1) -> bass.Bass:
    nc = bacc.Bacc()
    x_in = nc.declare_dram_parameter("x_in", [ROWS, M, L], dtype, isOutput=False)
    h_in = nc.declare_dram_parameter("h_in", [ROWS, M, S], dtype, isOutput=False)
    x0_out = nc.declare_dram_parameter("x0_out", [ROWS, M, L], dtype, isOutput=True)
    ss_out = nc.declare_dram_parameter("ss_out", [ROWS, 1], _F32, isOutput=True)

    add = mybir.AluOpType.add

    with tile.TileContext(nc) as tc, ExitStack() as ctx:
        main = ctx.enter_context(tc.tile_pool(name="main", bufs=1))
        reps = ctx.enter_context(tc.tile_pool(name="reps", bufs=rep_bufs))
        tmps = ctx.enter_context(tc.tile_pool(name="tmps", bufs=tmp_bufs))

        xs = main.tile([ROWS, M, L], dtype, tag="xs")
        hs = main.tile([ROWS, M, S], dtype, tag="hs")
        ys = main.tile([ROWS, W, S], dtype, tag="ys")
        ysb = main.tile([ROWS, W, S], dtype, tag="ysb")
        x0 = main.tile([ROWS, M, L], dtype, tag="x0")
        ss = main.tile([ROWS, 1], _F32, tag="ss")

        nc.sync.dma_start(out=hs, in_=h_in[:])
        nc.sync.dma_start(out=xs, in_=x_in[:])
        # ys gets a direct write for l=0 over w in [0, M); only its tail needs
        # zeroing.  ysb (the GPSIMD-side accumulator) is zeroed in full.
        nc.gpsimd.memset(ys[:, M:, :], 0.0)
        if gps_s2 or gps_indep_s2:
            fg = L - gps_s2 - gps_indep_s2
            nc.gpsimd.memset(ysb[:, 0:fg, :], 0.0)
            nc.gpsimd.memset(ysb[:, fg + M :, :], 0.0)

        def x_bcast(l: int) -> bass.AP:
            # x[:, :, l] broadcast along a trailing s axis: [ROWS, M, S]
            sl = xs[:, :, l]
            return bass.AP(
                tensor=sl.tensor, offset=sl.offset, ap=[sl.ap[0], sl.ap[1], [0, S]]
            )

        # Stage 1+2: Y[p, n+l, s] += H[p, n, s] * x[p, n, l]
        # ScalarE materializes the broadcast so VectorE's multiply keeps
        # step-1 fp16 operands (packed 2x mode).  The l-accumulation is split
        # across two buffers so VectorE and GPSIMD own independent chains.
        GPS_S2 = set(range(L - gps_s2, L)) if gps_s2 else set()
        GPS_I2 = set(range(L - gps_s2 - gps_indep_s2, L - gps_s2)) if gps_indep_s2 else set()
        g2tmps = ctx.enter_context(tc.tile_pool(name="g2tmps", bufs=g2tmp_bufs)) if (gps_indep_s2 or gps_s2) else None
        first_gps = min(GPS_S2 | GPS_I2) if (GPS_S2 or GPS_I2) else None
        # interleave GPSIMD l's through the emission order so their ScalarE
        # broadcasts neither starve the VectorE stream nor arrive too late
        gps_ls = sorted(GPS_S2 | GPS_I2)
        dve_s2 = [l for l in range(L) if l not in GPS_S2 and l not in GPS_I2]
        s2_order = []
        gi = 0
        for idx, l in enumerate(dve_s2):
            s2_order.append(l)
            if gi < len(gps_ls) and idx % s2_stride == s2_stride - 1:
                s2_order.append(gps_ls[gi])
                gi += 1
        s2_order.extend(gps_ls[gi:])
        for l in s2_order:
            on_gps = l in GPS_S2 or l in GPS_I2
            if l == 0:
                # direct broadcast read (1x mode) — slower per element but
                # starts as soon as the input DMAs land, before ScalarE's
                # first broadcast copy would be ready
                nc.vector.tensor_mul(out=ys[:, 0:M, :], in0=hs, in1=x_bcast(0))
                continue
            if on_gps:
                # GPSIMD has no packed perf modes, so its multiplies read the
                # step-0 broadcast AP directly — no ScalarE copy needed.
                if l == first_gps:
                    # first GPSIMD l writes ysb directly (no add needed)
                    nc.gpsimd.tensor_mul(
                        out=ysb[:, l : l + M, :], in0=hs, in1=x_bcast(l)
                    )
                else:
                    tmp = g2tmps.tile([ROWS, M, S], dtype, tag="g2tmp")
                    nc.gpsimd.tensor_mul(out=tmp, in0=hs, in1=x_bcast(l))
                    ysl = ysb[:, l : l + M, :]
                    nc.gpsimd.tensor_add(out=ysl, in0=ysl, in1=tmp)
            elif s2_chunk and l >= s2_chunk_from and (l - s2_chunk_from) % 2 == 0 and l + 1 in dve_s2:
                # paired: two ScalarE copies into one double tile, ONE multiply
                xr = reps.tile([ROWS, 2, M, S], dtype, tag="xr")
                nc.scalar.copy(out=xr[:, 0], in_=x_bcast(l))
                nc.scalar.copy(out=xr[:, 1], in_=x_bcast(l + 1))
                tmp = tmps.tile([ROWS, 2, M, S], dtype, tag="tmp")
                nc.vector.tensor_mul(
                    out=tmp,
                    in0=bass.AP(
                        tensor=hs.tensor,
                        offset=hs.offset,
                        ap=[hs.ap[0], [0, 2], [S, M], [1, S]],
                    ),
                    in1=xr,
                )
                for k in range(2):
                    ysl = ys[:, l + k : l + k + M, :]
                    nc.vector.tensor_add(out=ysl, in0=ysl, in1=tmp[:, k])
            elif s2_chunk and l >= s2_chunk_from and (l - s2_chunk_from) % 2 == 1:
                continue  # consumed by the pair above
            else:
                xr = reps.tile([ROWS, M, S], dtype, tag="xr")
                nc.scalar.copy(out=xr, in_=x_bcast(l))
                tmp = tmps.tile([ROWS, M, S], dtype, tag="tmp")
                nc.vector.tensor_mul(out=tmp, in0=hs, in1=xr)
                ysl = ys[:, l : l + M, :]
                nc.vector.tensor_add(out=ysl, in0=ysl, in1=tmp)
        if GPS_S2 or GPS_I2:
            # merge the two accumulators
            merge_eng = nc.gpsimd if gps_merge else nc.vector
            merge_eng.tensor_add(out=ys, in0=ys, in1=ysb)

        # Stage 3 partial: per-partition sum(Y^2) via ScalarE Square+accumulate.
        # ysb is dead after the merge, so it doubles as the Square write target.
        nc.scalar.activation(
            out=ysb, in_=ys, func=mybir.ActivationFunctionType.Square, accum_out=ss
        )
        nc.sync.dma_start(out=ss_out[:], in_=ss)

        # Stage 4: X0[p, n, l] = sum_s H[p, n, s] * Y[p, n+l, s]
        # s-contraction as an aligned fold tree: 22 -> 16 -> 8 -> 4 -> 2 -> 1
        # VectorE does all multiplies; fold chains are split VectorE/GPSIMD.
        FOLDS = ((0, 16, 6), (0, 8, 8), (0, 4, 4), (0, 2, 2))
        GPS_I4 = set(range(L - gps_indep_s4, L)) if gps_indep_s4 else set()
        # handoff l's: VectorE does the multiply, GPSIMD the fold chain
        GPS_H4 = (
            set(range(L - gps_indep_s4 - gps_hand_s4, L - gps_indep_s4))
            if gps_hand_s4
            else set()
        )
        gpool = ctx.enter_context(tc.tile_pool(name="gpool", bufs=gpool_bufs)) if (gps_indep_s4 or gps_indep_s2) else None
        dve_ls = [l for l in range(L) if l not in GPS_I4 and l not in GPS_H4]

        def ap3(t, pair_step, pairs, d1_step, d1_n, d2_step, d2_n, off):
            return bass.AP(
                tensor=t.tensor,
                offset=t.offset + off,
                ap=[t.ap[0], [pair_step, pairs], [d1_step, d1_n], [d2_step, d2_n]],
            )

        # VectorE side: pair-batched pipelines (one mul + one fold tree per
        # two l values, strided across the pair axis of a double-wide tile).
        i = 0
        while i < len(dve_ls):
            l = dve_ls[i]
            npair = 1
            while (
                npair < s4_chunk
                and i + npair < len(dve_ls)
                and dve_ls[i + npair] == l + npair
            ):
                npair += 1
            i += npair
            tmp = tmps.tile([ROWS, npair, M, S], dtype, tag="tmp")
            nc.vector.tensor_mul(
                out=tmp,
                in0=ap3(hs, 0, npair, S, M, 1, S, 0),
                in1=ap3(ys, S, npair, S, M, 1, S, l * S),
            )
            for dst, src, width in FOLDS:
                o = ap3(tmp, M * S, npair, S, M, 1, width, dst)
                nc.vector.tensor_tensor(
                    out=o,
                    in0=o,
                    in1=ap3(tmp, M * S, npair, S, M, 1, width, src),
                    op=add,
                )
            nc.vector.tensor_tensor(
                out=bass.AP(
                    tensor=x0.tensor,
                    offset=x0.offset + l,
                    ap=[x0.ap[0], [1, npair], [L, M]],
                ),
                in0=ap3(tmp, M * S, npair, S, M, 1, 1, 0)[:, :, :, 0],
                in1=ap3(tmp, M * S, npair, S, M, 1, 1, 1)[:, :, :, 0],
                op=add,
            )
        # GPSIMD side: independent single-l pipelines (plus handoff l's whose
        # multiply ran on VectorE).
        for l in sorted(GPS_I4 | GPS_H4):
            tmp = gpool.tile([ROWS, M, S], dtype, tag="gtmp")
            mul_eng = nc.vector if l in GPS_H4 else nc.gpsimd
            mul_eng.tensor_mul(out=tmp, in0=hs, in1=ys[:, l : l + M, :])
            for dst, src, width in FOLDS:
                o = tmp[:, :, dst : dst + width]
                nc.gpsimd.tensor_tensor(
                    out=o, in0=o, in1=tmp[:, :, src : src + width], op=add
                )
            nc.gpsimd.tensor_tensor(
                out=x0[:, :, l], in0=tmp[:, :, 0], in1=tmp[:, :, 1], op=add
            )
        nc.sync.dma_start(out=x0_out[:], in_=x0)

    nc.finalize()
    return nc


def shard_inputs(
    x: np.ndarray, H: np.ndarray, np_dtype=np.float16
) -> list[dict[str, np.ndarray]]:
    H2 = H[0, :, :, 0, :]  # (M, M, S)
    x = x.astype(np_dtype)
    H2 = H2.astype(np_dtype)
    in_maps = []
    for c in range(N_CORES):
        b, half = c // 2, c % 2
        m0 = half * ROWS
        in_maps.append(
            {
                "x_in": np.ascontiguousarray(x[b, m0 : m0 + ROWS]),
                "h_in": np.ascontiguousarray(H2[m0 : m0 + ROWS]),
            }
        )
    return in_maps


def finalize(
    results: list[dict[str, np.ndarray]],
    H: np.ndarray,
    noise_eps: np.ndarray,
) -> np.ndarray:
    X0 = np.empty((B, M, M, L), np.float32)
    sumsq = 0.0
    for c in range(N_CORES):
        b, half = c // 2, c % 2
        m0 = half * ROWS
        X0[b, m0 : m0 + ROWS] = results[c]["x0_out"].astype(np.float32)
        sumsq += results[c]["ss_out"].sum(dtype=np.float64)
    sigm = sumsq / (M * W * B * 10.0 ** (NOISE_DB / 10.0))

    H2 = H[0, :, :, 0, :]  # (M, M, S)
    hsum = H2.sum(axis=-1)  # (M, M)
    # noise window: nwin[b, m, n, l] = noise_eps[b, m, n + l, 0]
    nwin = np.lib.stride_tricks.sliding_window_view(noise_eps[:, :, :, 0], L, axis=2)
    X = X0 + np.float32(np.sqrt(sigm)) * (hsum[None, :, :, None] * nwin)
    X = X.astype(np.float32, copy=False)
    return X / X.max()


_NC_CACHE: bass.Bass | None = None


def kernel(x: np.ndarray, H: np.ndarray, noise_eps: np.ndarray) -> np.ndarray:
    global _NC_CACHE
    x = np.asarray(x, dtype=np.float32)
    H = np.asarray(H, dtype=np.float32)
    noise_eps = np.asarray(noise_eps, dtype=np.float32)
    if _NC_CACHE is None:
        _NC_CACHE = build_bass()
    in_maps = shard_inputs(x, H)
    res = run_bass_kernel_spmd(_NC_CACHE, in_maps, core_ids=list(range(N_CORES)))
    return finalize(res.results, H, noise_eps)



# revision 9
# speedup vs baseline: 1.0225x; 1.0225x over previous
"""CASSI layer kernel for Trainium2 (8 NeuronCores, Bass/Tile).

Math (matches the reference nn_CASSI_layer):
    H2[m,n,s]        = H[0,m,n,0,s]
    Y[b,m,n+l,s]    += H2[m,n,s] * x[b,m,n,l]            (shear-sum, l in [0,24))
    sigm             = sum(Y^2) / (M*W*B*10^(40/10))
    Yn               = Y + sqrt(sigm) * noise_eps         (noise_eps broadcast over s)
    X[b,m,n,l]       = sum_s H2[m,n,s] * Yn[b,m,n+l,s]
    out              = X / max(X)

Distribution: the (b, m) pairs form 4*256 = 1024 independent rows; each of the
8 cores takes 128 rows (core c: b = c//2, m in [128*(c%2), 128*(c%2)+128)),
mapped onto the 128 SBUF partitions.  Everything per-row lives along the free
dimension, so the spectral shifts are plain address offsets (always 4-byte
aligned in fp16 because the shift stride is S=22 elements).

The two global scalar couplings (sigm, max) are linearized out of the device
kernel: X = X0 + sqrt(sigm)*Xn with X0 the noise-free result (device) and
Xn[b,m,n,l] = (sum_s H2[m,n,s]) * noise_eps[b,m,n+l] (cheap host outer
product).  The device returns X0 and per-partition sum(Y^2); the host applies
sigma, the noise term, and the global max normalization.

Engine split per core: ScalarE materializes the x-column broadcasts over the
s axis; VectorE runs fp16 multiplies/adds in the packed 2x perf mode (the
shear offsets l*S*2 bytes are all 4-byte aligned, and stage-4 pipelines are
pair-batched over l to amortize per-op overhead); GPSIMD owns independent
pipelines for the last few l values in both stages (a second Y accumulator in
stage 2, full mul+fold chains in stage 4); the s-contraction is a
22->16->8->4->2->1 aligned fold tree; and sum(Y^2) rides the ScalarE Square
activation's accumulator.  GPSIMD multiplies read the step-0 broadcast APs
directly (it has no packed perf modes to forfeit), so its chains start right
after the input DMAs with no ScalarE dependency; the first VectorE multiply
likewise runs 1x off the broadcast to skip the ScalarE ramp.  Engine
assignments were tuned with the calibrated instruction-cost timeline
simulator (316us all-VectorE -> 250.6us final; deeper GPSIMD assignment,
cross-engine fold handoffs, emission reorders, strided DMA prefetch/split,
and quad-chunking all measured worse, leaving VectorE's minimal stream --
stage-2 muls+adds, accumulator merge, stage-4 muls+fold trees -- as the
critical path, balanced within ~10us of the GPSIMD chains).
"""

from contextlib import ExitStack

import numpy as np

import concourse.bass as bass
import concourse.bacc as bacc
import concourse.tile as tile
from concourse import mybir
from concourse.bass_utils import run_bass_kernel_spmd

B, M, L, S = 4, 256, 24, 22
W = M + L - 1  # 279
N_CORES = 8
ROWS = 128  # (b, m) rows per core
NOISE_DB = 40.0

_F32 = mybir.dt.float32
_F16 = mybir.dt.float16


def build_bass(dtype=_F16, gps_s2=5, gps_hand_s4=0, tmp_bufs=24, rep_bufs=3, gps_indep_s4=5, n_pass=6, psum_bufs=8, s4_chunk=2, wchunk=23) -> bass.Bass:
    nc = bacc.Bacc()
    x_in = nc.declare_dram_parameter("x_in", [ROWS, M, L], dtype, isOutput=False)
    h_in = nc.declare_dram_parameter("h_in", [ROWS, M, S], dtype, isOutput=False)
    id_in = nc.declare_dram_parameter("id_in", [ROWS, ROWS], dtype, isOutput=False)
    x0_out = nc.declare_dram_parameter("x0_out", [ROWS, M, L], dtype, isOutput=True)
    ss_out = nc.declare_dram_parameter("ss_out", [ROWS, 1], _F32, isOutput=True)

    add = mybir.AluOpType.add

    with tile.TileContext(nc) as tc, ExitStack() as ctx:
        main = ctx.enter_context(tc.tile_pool(name="main", bufs=1))
        reps = ctx.enter_context(tc.tile_pool(name="reps", bufs=rep_bufs))
        tmps = ctx.enter_context(tc.tile_pool(name="tmps", bufs=2))
        s2tmps = ctx.enter_context(tc.tile_pool(name="s2tmps", bufs=tmp_bufs))
        ypool = ctx.enter_context(tc.tile_pool(name="ypool", bufs=psum_bufs, space="PSUM"))

        xs = main.tile([ROWS, M, L], dtype, tag="xs")
        hs = main.tile([ROWS, M, S], dtype, tag="hs")
        ident = main.tile([ROWS, ROWS], dtype, tag="ident")
        ys = main.tile([ROWS, W, S], dtype, tag="ys")
        x0 = main.tile([ROWS, M, L], dtype, tag="x0")
        ss = main.tile([ROWS, 1], _F32, tag="ss")

        nc.sync.dma_start(out=hs, in_=h_in[:])
        nc.sync.dma_start(out=xs, in_=x_in[:])
        nc.sync.dma_start(out=ident, in_=id_in[:])

        def x_bcast(l: int, n0: int = 0, n1: int = M) -> bass.AP:
            # x[:, n0:n1, l] broadcast along a trailing s axis: [ROWS, n1-n0, S]
            sl = xs[:, n0:n1, l]
            return bass.AP(
                tensor=sl.tensor, offset=sl.offset, ap=[sl.ap[0], sl.ap[1], [0, S]]
            )

        # Stage 1+2: Y[p, w, s] = sum_l H[p, w-l, s] * x[p, w-l, l]
        # DVE/GPSIMD compute the per-l products HX_l (DVE from a ScalarE-
        # materialized broadcast, packed 2x mode; GPSIMD straight off the
        # broadcast AP); the TensorEngine does the shift-accumulate with
        # identity-weight matmuls into PSUM banks (fp32), and ScalarE
        # flushes each finished bank into the fp16 ys tile.  The w axis is
        # processed in n_pass passes so only a pass's worth of HX_l slices
        # is alive in SBUF at a time.
        GPS_S2 = set(range(L - gps_s2, L)) if gps_s2 else set()
        # w-chunk boundaries: bank-sized pieces of the W axis
        bounds = list(range(0, W, wchunk)) + [W]
        chunks = [(bounds[i], bounds[i + 1]) for i in range(len(bounds) - 1)]
        # group chunks into passes of roughly equal count
        per = (len(chunks) + n_pass - 1) // n_pass
        passes = [chunks[i : i + per] for i in range(0, len(chunks), per)]
        for pchunks in passes:
            P0, P1 = pchunks[0][0], pchunks[-1][1]
            # products for this pass: HX_l[n] for n in [max(0,P0-l), min(M,P1-l))
            ptmp = {}
            for l in range(L):
                nb0, nb1 = max(0, P0 - l), min(M, P1 - l)
                if nb0 >= nb1:
                    continue
                tmp = s2tmps.tile([ROWS, nb1 - nb0, S], dtype, tag="s2tmp")
                ptmp[l] = (tmp, nb0)
                if l in GPS_S2:
                    nc.gpsimd.tensor_mul(out=tmp, in0=hs[:, nb0:nb1, :], in1=x_bcast(l, nb0, nb1))
                elif l == 0:
                    # direct broadcast read (1x mode): starts without waiting
                    # for ScalarE's first broadcast copy
                    nc.vector.tensor_mul(out=tmp, in0=hs[:, nb0:nb1, :], in1=x_bcast(l, nb0, nb1))
                else:
                    xr = reps.tile([ROWS, nb1 - nb0, S], dtype, tag="xr")
                    nc.scalar.copy(out=xr, in_=x_bcast(l, nb0, nb1))
                    nc.vector.tensor_mul(out=tmp, in0=hs[:, nb0:nb1, :], in1=xr)
            for (W0, W1) in pchunks:
                ls = [l for l in range(L) if max(W0, l) < min(W1, l + M)]
                ybank = ypool.tile([ROWS, W1 - W0, S], _F32, tag="ybank")
                for i, l in enumerate(ls):
                    wa, wb = max(W0, l), min(W1, l + M)
                    tmp, nb0 = ptmp[l]
                    nc.tensor.matmul(
                        out=ybank[:, wa - W0 : wb - W0, :],
                        lhsT=ident,
                        rhs=tmp[:, wa - l - nb0 : wb - l - nb0, :],
                        start=(i == 0),
                        stop=(i == len(ls) - 1),
                        skip_group_check=True,
                    )
                nc.scalar.copy(out=ys[:, W0:W1, :], in_=ybank)

        # Stage 3 partial: per-partition sum(Y^2) via ScalarE Square+accumulate.
        # x0 is not written until stage 4, so it doubles as the Square target
        # (ys is [W,S] = 6138 elems; x0 is [M,L] = 6144 — use a flat slice).
        sq = bass.AP(tensor=x0.tensor, offset=x0.offset, ap=[x0.ap[0], [1, W * S]])
        ysf = bass.AP(tensor=ys.tensor, offset=ys.offset, ap=[ys.ap[0], [1, W * S]])
        nc.scalar.activation(
            out=sq, in_=ysf, func=mybir.ActivationFunctionType.Square, accum_out=ss
        )
        nc.sync.dma_start(out=ss_out[:], in_=ss)

        # Stage 4: X0[p, n, l] = sum_s H[p, n, s] * Y[p, n+l, s]
        # s-contraction as an aligned fold tree: 22 -> 16 -> 8 -> 4 -> 2 -> 1
        # VectorE does all multiplies; fold chains are split VectorE/GPSIMD.
        FOLDS = ((0, 16, 6), (0, 8, 8), (0, 4, 4), (0, 2, 2))
        GPS_I4 = set(range(L - gps_indep_s4, L)) if gps_indep_s4 else set()
        # handoff l's: VectorE does the multiply, GPSIMD the fold chain
        GPS_H4 = (
            set(range(L - gps_indep_s4 - gps_hand_s4, L - gps_indep_s4))
            if gps_hand_s4
            else set()
        )
        gpool = ctx.enter_context(tc.tile_pool(name="gpool", bufs=2)) if (gps_indep_s4 or gps_hand_s4) else None
        dve_ls = [l for l in range(L) if l not in GPS_I4 and l not in GPS_H4]

        def ap3(t, pair_step, pairs, d1_step, d1_n, d2_step, d2_n, off):
            return bass.AP(
                tensor=t.tensor,
                offset=t.offset + off,
                ap=[t.ap[0], [pair_step, pairs], [d1_step, d1_n], [d2_step, d2_n]],
            )

        # VectorE side: pair-batched pipelines (one mul + one fold tree per
        # two l values, strided across the pair axis of a double-wide tile).
        i = 0
        while i < len(dve_ls):
            l = dve_ls[i]
            npair = 1
            while (
                npair < s4_chunk
                and i + npair < len(dve_ls)
                and dve_ls[i + npair] == l + npair
            ):
                npair += 1
            i += npair
            tmp = tmps.tile([ROWS, npair, M, S], dtype, tag="tmp")
            nc.vector.tensor_mul(
                out=tmp,
                in0=ap3(hs, 0, npair, S, M, 1, S, 0),
                in1=ap3(ys, S, npair, S, M, 1, S, l * S),
            )
            for dst, src, width in FOLDS:
                o = ap3(tmp, M * S, npair, S, M, 1, width, dst)
                nc.vector.tensor_tensor(
                    out=o,
                    in0=o,
                    in1=ap3(tmp, M * S, npair, S, M, 1, width, src),
                    op=add,
                )
            nc.vector.tensor_tensor(
                out=bass.AP(
                    tensor=x0.tensor,
                    offset=x0.offset + l,
                    ap=[x0.ap[0], [1, npair], [L, M]],
                ),
                in0=ap3(tmp, M * S, npair, S, M, 1, 1, 0)[:, :, :, 0],
                in1=ap3(tmp, M * S, npair, S, M, 1, 1, 1)[:, :, :, 0],
                op=add,
            )
        # GPSIMD side: independent single-l pipelines (plus handoff l's whose
        # multiply ran on VectorE).
        for l in sorted(GPS_I4 | GPS_H4):
            tmp = gpool.tile([ROWS, M, S], dtype, tag="gtmp")
            mul_eng = nc.vector if l in GPS_H4 else nc.gpsimd
            mul_eng.tensor_mul(out=tmp, in0=hs, in1=ys[:, l : l + M, :])
            for dst, src, width in FOLDS:
                o = tmp[:, :, dst : dst + width]
                nc.gpsimd.tensor_tensor(
                    out=o, in0=o, in1=tmp[:, :, src : src + width], op=add
                )
            nc.gpsimd.tensor_tensor(
                out=x0[:, :, l], in0=tmp[:, :, 0], in1=tmp[:, :, 1], op=add
            )
        nc.sync.dma_start(out=x0_out[:], in_=x0)

    nc.finalize()
    return nc


def shard_inputs(
    x: np.ndarray, H: np.ndarray, np_dtype=np.float16
) -> list[dict[str, np.ndarray]]:
    H2 = H[0, :, :, 0, :]  # (M, M, S)
    x = x.astype(np_dtype)
    H2 = H2.astype(np_dtype)
    ident = np.eye(ROWS, dtype=np_dtype)
    in_maps = []
    for c in range(N_CORES):
        b, half = c // 2, c % 2
        m0 = half * ROWS
        in_maps.append(
            {
                "x_in": np.ascontiguousarray(x[b, m0 : m0 + ROWS]),
                "h_in": np.ascontiguousarray(H2[m0 : m0 + ROWS]),
                "id_in": ident,
            }
        )
    return in_maps


def finalize(
    results: list[dict[str, np.ndarray]],
    H: np.ndarray,
    noise_eps: np.ndarray,
) -> np.ndarray:
    X0 = np.empty((B, M, M, L), np.float32)
    sumsq = 0.0
    for c in range(N_CORES):
        b, half = c // 2, c % 2
        m0 = half * ROWS
        X0[b, m0 : m0 + ROWS] = results[c]["x0_out"].astype(np.float32)
        sumsq += results[c]["ss_out"].sum(dtype=np.float64)
    sigm = sumsq / (M * W * B * 10.0 ** (NOISE_DB / 10.0))

    H2 = H[0, :, :, 0, :]  # (M, M, S)
    hsum = H2.sum(axis=-1)  # (M, M)
    # noise window: nwin[b, m, n, l] = noise_eps[b, m, n + l, 0]
    nwin = np.lib.stride_tricks.sliding_window_view(noise_eps[:, :, :, 0], L, axis=2)
    X = X0 + np.float32(np.sqrt(sigm)) * (hsum[None, :, :, None] * nwin)
    X = X.astype(np.float32, copy=False)
    return X / X.max()


_NC_CACHE: bass.Bass | None = None


def kernel(x: np.ndarray, H: np.ndarray, noise_eps: np.ndarray) -> np.ndarray:
    global _NC_CACHE
    x = np.asarray(x, dtype=np.float32)
    H = np.asarray(H, dtype=np.float32)
    noise_eps = np.asarray(noise_eps, dtype=np.float32)
    if _NC_CACHE is None:
        _NC_CACHE = build_bass()
    in_maps = shard_inputs(x, H)
    res = run_bass_kernel_spmd(_NC_CACHE, in_maps, core_ids=list(range(N_CORES)))
    return finalize(res.results, H, noise_eps)



# revision 12
# speedup vs baseline: 1.1593x; 1.1338x over previous
"""CASSI layer kernel for Trainium2 (8 NeuronCores, Bass/Tile).

Math (matches the reference nn_CASSI_layer):
    H2[m,n,s]        = H[0,m,n,0,s]
    Y[b,m,n+l,s]    += H2[m,n,s] * x[b,m,n,l]            (shear-sum, l in [0,24))
    sigm             = sum(Y^2) / (M*W*B*10^(40/10))
    Yn               = Y + sqrt(sigm) * noise_eps         (noise_eps broadcast over s)
    X[b,m,n,l]       = sum_s H2[m,n,s] * Yn[b,m,n+l,s]
    out              = X / max(X)

Distribution: the (b, m) pairs form 4*256 = 1024 independent rows; each of the
8 cores takes 128 rows (core c: b = c//2, m in [128*(c%2), 128*(c%2)+128)),
mapped onto the 128 SBUF partitions.  Everything per-row lives along the free
dimension, so the spectral shifts are plain address offsets (always 4-byte
aligned in fp16 because the shift stride is S=22 elements).

The two global scalar couplings (sigm, max) are linearized out of the device
kernel: X = X0 + sqrt(sigm)*Xn with X0 the noise-free result (device) and
Xn[b,m,n,l] = (sum_s H2[m,n,s]) * noise_eps[b,m,n+l] (cheap host outer
product).  The device returns X0 and per-partition sum(Y^2); the host applies
sigma, the noise term, and the global max normalization.

Engine split per core: ScalarE materializes the x-column broadcasts over the
s axis; VectorE runs fp16 multiplies/adds in the packed 2x perf mode (the
shear offsets l*S*2 bytes are all 4-byte aligned, and stage-4 pipelines are
pair-batched over l to amortize per-op overhead); GPSIMD owns independent
pipelines for the last few l values in both stages (a second Y accumulator in
stage 2, full mul+fold chains in stage 4); the s-contraction is a
22->16->8->4->2->1 aligned fold tree; and sum(Y^2) rides the ScalarE Square
activation's accumulator.  GPSIMD multiplies read the step-0 broadcast APs
directly (it has no packed perf modes to forfeit), so its chains start right
after the input DMAs with no ScalarE dependency; the first VectorE multiply
likewise runs 1x off the broadcast to skip the ScalarE ramp.  Engine
assignments were tuned with the calibrated instruction-cost timeline
simulator (316us all-VectorE -> 250.6us final; deeper GPSIMD assignment,
cross-engine fold handoffs, emission reorders, strided DMA prefetch/split,
and quad-chunking all measured worse, leaving VectorE's minimal stream --
stage-2 muls+adds, accumulator merge, stage-4 muls+fold trees -- as the
critical path, balanced within ~10us of the GPSIMD chains).
"""

from contextlib import ExitStack

import numpy as np

import concourse.bass as bass
import concourse.bacc as bacc
import concourse.tile as tile
from concourse import mybir
from concourse.bass_utils import run_bass_kernel_spmd

B, M, L, S = 4, 256, 24, 22
W = M + L - 1  # 279
N_CORES = 8
ROWS = 128  # (b, m) rows per core
NOISE_DB = 40.0

_F32 = mybir.dt.float32
_F16 = mybir.dt.float16


def build_bass(dtype=_F16, gps_s2=7, gps_hand_s4=0, tmp_bufs=24, rep_bufs=5, gps_indep_s4=5, n_pass=6, psum_bufs=8, s4_chunk=2, wchunk=22, direct_stride=3) -> bass.Bass:
    nc = bacc.Bacc()
    x_in = nc.declare_dram_parameter("x_in", [ROWS, M, L], dtype, isOutput=False)
    h_in = nc.declare_dram_parameter("h_in", [ROWS, M, S], dtype, isOutput=False)
    id_in = nc.declare_dram_parameter("id_in", [ROWS, ROWS], dtype, isOutput=False)
    x0_out = nc.declare_dram_parameter("x0_out", [ROWS, M, L], dtype, isOutput=True)
    ss_out = nc.declare_dram_parameter("ss_out", [ROWS, 1], _F32, isOutput=True)

    add = mybir.AluOpType.add

    with tile.TileContext(nc) as tc, ExitStack() as ctx:
        main = ctx.enter_context(tc.tile_pool(name="main", bufs=1))
        reps = ctx.enter_context(tc.tile_pool(name="reps", bufs=rep_bufs))
        tmps = ctx.enter_context(tc.tile_pool(name="tmps", bufs=2))
        s2tmps = ctx.enter_context(tc.tile_pool(name="s2tmps", bufs=tmp_bufs))
        ypool = ctx.enter_context(tc.tile_pool(name="ypool", bufs=psum_bufs, space="PSUM"))

        xs = main.tile([ROWS, M, L], dtype, tag="xs")
        hs = main.tile([ROWS, M, S], dtype, tag="hs")
        ident = main.tile([ROWS, ROWS], dtype, tag="ident")
        ys = main.tile([ROWS, W, S], dtype, tag="ys")
        x0 = main.tile([ROWS, M, L], dtype, tag="x0")
        ss = main.tile([ROWS, 1], _F32, tag="ss")

        nc.sync.dma_start(out=hs, in_=h_in[:])
        nc.sync.dma_start(out=xs, in_=x_in[:])
        nc.sync.dma_start(out=ident, in_=id_in[:])

        def x_bcast(l: int, n0: int = 0, n1: int = M) -> bass.AP:
            # x[:, n0:n1, l] broadcast along a trailing s axis: [ROWS, n1-n0, S]
            sl = xs[:, n0:n1, l]
            return bass.AP(
                tensor=sl.tensor, offset=sl.offset, ap=[sl.ap[0], sl.ap[1], [0, S]]
            )

        # Stage 1+2: Y[p, w, s] = sum_l H[p, w-l, s] * x[p, w-l, l]
        # DVE/GPSIMD compute the per-l products HX_l (DVE from a ScalarE-
        # materialized broadcast, packed 2x mode; GPSIMD straight off the
        # broadcast AP); the TensorEngine does the shift-accumulate with
        # identity-weight matmuls into PSUM banks (fp32), and ScalarE
        # flushes each finished bank into the fp16 ys tile.  The w axis is
        # processed in n_pass passes so only a pass's worth of HX_l slices
        # is alive in SBUF at a time.
        GPS_S2 = set(range(L - gps_s2, L)) if gps_s2 else set()
        # w-chunk boundaries: bank-sized pieces of the W axis
        bounds = list(range(0, W, wchunk)) + [W]
        chunks = [(bounds[i], bounds[i + 1]) for i in range(len(bounds) - 1)]
        # group chunks into passes of roughly equal count
        per = (len(chunks) + n_pass - 1) // n_pass
        passes = [chunks[i : i + per] for i in range(0, len(chunks), per)]
        for pchunks in passes:
            P0, P1 = pchunks[0][0], pchunks[-1][1]
            # products for this pass: HX_l[n] for n in [max(0,P0-l), min(M,P1-l))
            ptmp = {}
            for l in range(L):
                nb0, nb1 = max(0, P0 - l), min(M, P1 - l)
                if nb0 >= nb1:
                    continue
                tmp = s2tmps.tile([ROWS, nb1 - nb0, S], dtype, tag="s2tmp")
                ptmp[l] = (tmp, nb0)
                if l in GPS_S2:
                    nc.gpsimd.tensor_mul(out=tmp, in0=hs[:, nb0:nb1, :], in1=x_bcast(l, nb0, nb1))
                elif l == 0 or (direct_stride and l % direct_stride == direct_stride - 1):
                    # direct broadcast read (1x mode): slower on DVE but needs
                    # no ScalarE copy — used to keep ScalarE off the critical
                    # path of each pass
                    nc.vector.tensor_mul(out=tmp, in0=hs[:, nb0:nb1, :], in1=x_bcast(l, nb0, nb1))
                else:
                    xr = reps.tile([ROWS, nb1 - nb0, S], dtype, tag="xr")
                    nc.scalar.copy(out=xr, in_=x_bcast(l, nb0, nb1))
                    nc.vector.tensor_mul(out=tmp, in0=hs[:, nb0:nb1, :], in1=xr)
            for (W0, W1) in pchunks:
                ls = [l for l in range(L) if max(W0, l) < min(W1, l + M)]
                ybank = ypool.tile([ROWS, W1 - W0, S], _F32, tag="ybank")
                for i, l in enumerate(ls):
                    wa, wb = max(W0, l), min(W1, l + M)
                    tmp, nb0 = ptmp[l]
                    nc.tensor.matmul(
                        out=ybank[:, wa - W0 : wb - W0, :],
                        lhsT=ident,
                        rhs=tmp[:, wa - l - nb0 : wb - l - nb0, :],
                        start=(i == 0),
                        stop=(i == len(ls) - 1),
                        skip_group_check=True,
                    )
                nc.scalar.copy(out=ys[:, W0:W1, :], in_=ybank)

        # Stage 3 partial: per-partition sum(Y^2) via ScalarE Square+accumulate.
        # x0 is not written until stage 4, so it doubles as the Square target
        # (ys is [W,S] = 6138 elems; x0 is [M,L] = 6144 — use a flat slice).
        sq = bass.AP(tensor=x0.tensor, offset=x0.offset, ap=[x0.ap[0], [1, W * S]])
        ysf = bass.AP(tensor=ys.tensor, offset=ys.offset, ap=[ys.ap[0], [1, W * S]])
        nc.scalar.activation(
            out=sq, in_=ysf, func=mybir.ActivationFunctionType.Square, accum_out=ss
        )
        nc.sync.dma_start(out=ss_out[:], in_=ss)

        # Stage 4: X0[p, n, l] = sum_s H[p, n, s] * Y[p, n+l, s]
        # s-contraction as an aligned fold tree: 22 -> 16 -> 8 -> 4 -> 2 -> 1
        # VectorE does all multiplies; fold chains are split VectorE/GPSIMD.
        FOLDS = ((0, 16, 6), (0, 8, 8), (0, 4, 4), (0, 2, 2))
        GPS_I4 = set(range(L - gps_indep_s4, L)) if gps_indep_s4 else set()
        # handoff l's: VectorE does the multiply, GPSIMD the fold chain
        GPS_H4 = (
            set(range(L - gps_indep_s4 - gps_hand_s4, L - gps_indep_s4))
            if gps_hand_s4
            else set()
        )
        gpool = ctx.enter_context(tc.tile_pool(name="gpool", bufs=2)) if (gps_indep_s4 or gps_hand_s4) else None
        dve_ls = [l for l in range(L) if l not in GPS_I4 and l not in GPS_H4]

        def ap3(t, pair_step, pairs, d1_step, d1_n, d2_step, d2_n, off):
            return bass.AP(
                tensor=t.tensor,
                offset=t.offset + off,
                ap=[t.ap[0], [pair_step, pairs], [d1_step, d1_n], [d2_step, d2_n]],
            )

        # VectorE side: pair-batched pipelines (one mul + one fold tree per
        # two l values, strided across the pair axis of a double-wide tile).
        i = 0
        while i < len(dve_ls):
            l = dve_ls[i]
            npair = 1
            while (
                npair < s4_chunk
                and i + npair < len(dve_ls)
                and dve_ls[i + npair] == l + npair
            ):
                npair += 1
            i += npair
            tmp = tmps.tile([ROWS, npair, M, S], dtype, tag="tmp")
            nc.vector.tensor_mul(
                out=tmp,
                in0=ap3(hs, 0, npair, S, M, 1, S, 0),
                in1=ap3(ys, S, npair, S, M, 1, S, l * S),
            )
            for dst, src, width in FOLDS:
                o = ap3(tmp, M * S, npair, S, M, 1, width, dst)
                nc.vector.tensor_tensor(
                    out=o,
                    in0=o,
                    in1=ap3(tmp, M * S, npair, S, M, 1, width, src),
                    op=add,
                )
            nc.vector.tensor_tensor(
                out=bass.AP(
                    tensor=x0.tensor,
                    offset=x0.offset + l,
                    ap=[x0.ap[0], [1, npair], [L, M]],
                ),
                in0=ap3(tmp, M * S, npair, S, M, 1, 1, 0)[:, :, :, 0],
                in1=ap3(tmp, M * S, npair, S, M, 1, 1, 1)[:, :, :, 0],
                op=add,
            )
        # GPSIMD side: independent single-l pipelines (plus handoff l's whose
        # multiply ran on VectorE).
        for l in sorted(GPS_I4 | GPS_H4):
            tmp = gpool.tile([ROWS, M, S], dtype, tag="gtmp")
            mul_eng = nc.vector if l in GPS_H4 else nc.gpsimd
            mul_eng.tensor_mul(out=tmp, in0=hs, in1=ys[:, l : l + M, :])
            for dst, src, width in FOLDS:
                o = tmp[:, :, dst : dst + width]
                nc.gpsimd.tensor_tensor(
                    out=o, in0=o, in1=tmp[:, :, src : src + width], op=add
                )
            nc.gpsimd.tensor_tensor(
                out=x0[:, :, l], in0=tmp[:, :, 0], in1=tmp[:, :, 1], op=add
            )
        nc.sync.dma_start(out=x0_out[:], in_=x0)

    nc.finalize()
    return nc


def shard_inputs(
    x: np.ndarray, H: np.ndarray, np_dtype=np.float16
) -> list[dict[str, np.ndarray]]:
    H2 = H[0, :, :, 0, :]  # (M, M, S)
    x = x.astype(np_dtype)
    H2 = H2.astype(np_dtype)
    ident = np.eye(ROWS, dtype=np_dtype)
    in_maps = []
    for c in range(N_CORES):
        b, half = c // 2, c % 2
        m0 = half * ROWS
        in_maps.append(
            {
                "x_in": np.ascontiguousarray(x[b, m0 : m0 + ROWS]),
                "h_in": np.ascontiguousarray(H2[m0 : m0 + ROWS]),
                "id_in": ident,
            }
        )
    return in_maps


def finalize(
    results: list[dict[str, np.ndarray]],
    H: np.ndarray,
    noise_eps: np.ndarray,
) -> np.ndarray:
    X0 = np.empty((B, M, M, L), np.float32)
    sumsq = 0.0
    for c in range(N_CORES):
        b, half = c // 2, c % 2
        m0 = half * ROWS
        X0[b, m0 : m0 + ROWS] = results[c]["x0_out"].astype(np.float32)
        sumsq += results[c]["ss_out"].sum(dtype=np.float64)
    sigm = sumsq / (M * W * B * 10.0 ** (NOISE_DB / 10.0))

    H2 = H[0, :, :, 0, :]  # (M, M, S)
    hsum = H2.sum(axis=-1)  # (M, M)
    # noise window: nwin[b, m, n, l] = noise_eps[b, m, n + l, 0]
    nwin = np.lib.stride_tricks.sliding_window_view(noise_eps[:, :, :, 0], L, axis=2)
    X = X0 + np.float32(np.sqrt(sigm)) * (hsum[None, :, :, None] * nwin)
    X = X.astype(np.float32, copy=False)
    return X / X.max()


_NC_CACHE: bass.Bass | None = None


def kernel(x: np.ndarray, H: np.ndarray, noise_eps: np.ndarray) -> np.ndarray:
    global _NC_CACHE
    x = np.asarray(x, dtype=np.float32)
    H = np.asarray(H, dtype=np.float32)
    noise_eps = np.asarray(noise_eps, dtype=np.float32)
    if _NC_CACHE is None:
        _NC_CACHE = build_bass()
    in_maps = shard_inputs(x, H)
    res = run_bass_kernel_spmd(_NC_CACHE, in_maps, core_ids=list(range(N_CORES)))
    return finalize(res.results, H, noise_eps)



# revision 25
# speedup vs baseline: 1.1737x; 1.0124x over previous
"""CASSI layer kernel for Trainium2 (8 NeuronCores, Bass/Tile).

Math (matches the reference nn_CASSI_layer):
    H2[m,n,s]        = H[0,m,n,0,s]
    Y[b,m,n+l,s]    += H2[m,n,s] * x[b,m,n,l]            (shear-sum, l in [0,24))
    sigm             = sum(Y^2) / (M*W*B*10^(40/10))
    Yn               = Y + sqrt(sigm) * noise_eps         (noise_eps broadcast over s)
    X[b,m,n,l]       = sum_s H2[m,n,s] * Yn[b,m,n+l,s]
    out              = X / max(X)

Distribution: the (b, m) pairs form 4*256 = 1024 independent rows; each of the
8 cores takes 128 rows (core c: b = c//2, m in [128*(c%2), 128*(c%2)+128)),
mapped onto the 128 SBUF partitions.  Everything per-row lives along the free
dimension, so the spectral shifts are plain address offsets (always 4-byte
aligned in fp16 because the shift stride is S=22 elements).

The two global scalar couplings (sigm, max) are linearized out of the device
kernel: X = X0 + sqrt(sigm)*Xn with X0 the noise-free result (device) and
Xn[b,m,n,l] = (sum_s H2[m,n,s]) * noise_eps[b,m,n+l] (cheap host outer
product).  The device returns X0 and per-partition sum(Y^2); the host applies
sigma, the noise term, and the global max normalization.

Engine split per core: stage 2's shift-ACCUMULATE runs on the TensorEngine
as identity-weight matmuls (lhsT = I_128, a host-shipped constant) that add
each per-l product HX_l = H (.) x_l into PSUM banks (fp32, one 22-wide w
chunk per 2KB bank), with ScalarE flushing finished banks into the fp16 ys
tile; the w axis is swept in 6 passes so only one pass's HX_l slices live in
SBUF.  The HX_l products themselves go to VectorE (fp16 packed 2x off a
ScalarE-materialized broadcast; every 3rd l reads the stride-0 broadcast
directly at 1x to keep ScalarE's copies off the per-pass critical path) and
GPSIMD (last 7 l's, straight off the broadcast AP).  Stage 4 is unchanged:
VectorE/GPSIMD muls + 22->16->8->4->2->1 aligned fold trees, pair-batched;
sum(Y^2) rides the ScalarE Square activation's accumulator (x0 doubles as
the scratch target).  Moving the stage-2 adds to the otherwise-idle PE cut
VectorE's stream from ~226us to ~170us and removed the dual-accumulator
merge; tuned with the calibrated instruction-cost timeline simulator
(250.6us prior all-vector version -> 216.2us; PSUM fp32 accumulation also
halves the relative error vs the fp16 ys accumulator it replaced).
"""

from contextlib import ExitStack

import numpy as np

import concourse.bass as bass
import concourse.bacc as bacc
import concourse.tile as tile
from concourse import mybir
from concourse.bass_utils import run_bass_kernel_spmd

B, M, L, S = 4, 256, 24, 22
W = M + L - 1  # 279
N_CORES = 8
ROWS = 128  # (b, m) rows per core
NOISE_DB = 40.0

_F32 = mybir.dt.float32
_F16 = mybir.dt.float16


def build_bass(dtype=_F16, gps_s2=7, gps_hand_s4=0, tmp_bufs=24, rep_bufs=5, gps_indep_s4=5, n_pass=6, psum_bufs=8, s4_chunk=2, wchunk=22, direct_stride=3) -> bass.Bass:
    nc = bacc.Bacc()
    x_in = nc.declare_dram_parameter("x_in", [ROWS, M, L], dtype, isOutput=False)
    h_in = nc.declare_dram_parameter("h_in", [ROWS, M, S], dtype, isOutput=False)
    id_in = nc.declare_dram_parameter("id_in", [ROWS, ROWS], dtype, isOutput=False)
    x0_out = nc.declare_dram_parameter("x0_out", [ROWS, M, L], dtype, isOutput=True)
    ss_out = nc.declare_dram_parameter("ss_out", [ROWS, 1], _F32, isOutput=True)

    add = mybir.AluOpType.add

    with tile.TileContext(nc) as tc, ExitStack() as ctx:
        main = ctx.enter_context(tc.tile_pool(name="main", bufs=1))
        reps = ctx.enter_context(tc.tile_pool(name="reps", bufs=rep_bufs))
        tmps = ctx.enter_context(tc.tile_pool(name="tmps", bufs=2))
        s2tmps = ctx.enter_context(tc.tile_pool(name="s2tmps", bufs=tmp_bufs))
        ypool = ctx.enter_context(tc.tile_pool(name="ypool", bufs=psum_bufs, space="PSUM"))

        xs = main.tile([ROWS, M, L], dtype, tag="xs")
        hs = main.tile([ROWS, M, S], dtype, tag="hs")
        ident = main.tile([ROWS, ROWS], dtype, tag="ident")
        ys = main.tile([ROWS, W, S], dtype, tag="ys")
        x0 = main.tile([ROWS, M, L], dtype, tag="x0")
        ss = main.tile([ROWS, 1], _F32, tag="ss")

        # Split the input DMAs along n so the first pass's multiplies start
        # after ~1/4 of the transfer instead of all of it (the DMA engines
        # are serialized in the model; splitting only moves the ready-time
        # of the early rows forward).
        DMA_SPLIT = 4
        step = M // DMA_SPLIT
        for i in range(DMA_SPLIT):
            a, b = i * step, (i + 1) * step
            nc.sync.dma_start(out=hs[:, a:b, :], in_=h_in[:, a:b, :])
            nc.sync.dma_start(out=xs[:, a:b, :], in_=x_in[:, a:b, :])
        nc.sync.dma_start(out=ident, in_=id_in[:])

        def x_bcast(l: int, n0: int = 0, n1: int = M) -> bass.AP:
            # x[:, n0:n1, l] broadcast along a trailing s axis: [ROWS, n1-n0, S]
            sl = xs[:, n0:n1, l]
            return bass.AP(
                tensor=sl.tensor, offset=sl.offset, ap=[sl.ap[0], sl.ap[1], [0, S]]
            )

        # Stage 1+2: Y[p, w, s] = sum_l H[p, w-l, s] * x[p, w-l, l]
        # DVE/GPSIMD compute the per-l products HX_l (DVE from a ScalarE-
        # materialized broadcast, packed 2x mode; GPSIMD straight off the
        # broadcast AP); the TensorEngine does the shift-accumulate with
        # identity-weight matmuls into PSUM banks (fp32), and ScalarE
        # flushes each finished bank into the fp16 ys tile.  The w axis is
        # processed in n_pass passes so only a pass's worth of HX_l slices
        # is alive in SBUF at a time.
        GPS_S2 = set(range(L - gps_s2, L)) if gps_s2 else set()
        # w-chunk boundaries: bank-sized pieces of the W axis
        bounds = list(range(0, W, wchunk)) + [W]
        chunks = [(bounds[i], bounds[i + 1]) for i in range(len(bounds) - 1)]
        # group chunks into passes of roughly equal count
        per = (len(chunks) + n_pass - 1) // n_pass
        passes = [chunks[i : i + per] for i in range(0, len(chunks), per)]
        for pchunks in passes:
            P0, P1 = pchunks[0][0], pchunks[-1][1]
            # products for this pass: HX_l[n] for n in [max(0,P0-l), min(M,P1-l))
            ptmp = {}
            for l in range(L):
                nb0, nb1 = max(0, P0 - l), min(M, P1 - l)
                if nb0 >= nb1:
                    continue
                tmp = s2tmps.tile([ROWS, nb1 - nb0, S], dtype, tag="s2tmp")
                ptmp[l] = (tmp, nb0)
                if l in GPS_S2:
                    nc.gpsimd.tensor_mul(out=tmp, in0=hs[:, nb0:nb1, :], in1=x_bcast(l, nb0, nb1))
                elif l == 0 or (direct_stride and l % direct_stride == direct_stride - 1):
                    # direct broadcast read (1x mode): slower on DVE but needs
                    # no ScalarE copy — used to keep ScalarE off the critical
                    # path of each pass
                    nc.vector.tensor_mul(out=tmp, in0=hs[:, nb0:nb1, :], in1=x_bcast(l, nb0, nb1))
                else:
                    xr = reps.tile([ROWS, nb1 - nb0, S], dtype, tag="xr")
                    nc.scalar.copy(out=xr, in_=x_bcast(l, nb0, nb1))
                    nc.vector.tensor_mul(out=tmp, in0=hs[:, nb0:nb1, :], in1=xr)
            for (W0, W1) in pchunks:
                ls = [l for l in range(L) if max(W0, l) < min(W1, l + M)]
                ybank = ypool.tile([ROWS, W1 - W0, S], _F32, tag="ybank")
                for i, l in enumerate(ls):
                    wa, wb = max(W0, l), min(W1, l + M)
                    tmp, nb0 = ptmp[l]
                    nc.tensor.matmul(
                        out=ybank[:, wa - W0 : wb - W0, :],
                        lhsT=ident,
                        rhs=tmp[:, wa - l - nb0 : wb - l - nb0, :],
                        start=(i == 0),
                        stop=(i == len(ls) - 1),
                        skip_group_check=True,
                    )
                nc.scalar.copy(out=ys[:, W0:W1, :], in_=ybank)

        # Stage 3 partial: per-partition sum(Y^2) via ScalarE Square+accumulate.
        # x0 is not written until stage 4's pipelines finish their folds, so
        # it doubles as the Square target (flat [W*S] slice of its [M*L]).
        sq = bass.AP(tensor=x0.tensor, offset=x0.offset, ap=[x0.ap[0], [1, W * S]])
        ysf = bass.AP(tensor=ys.tensor, offset=ys.offset, ap=[ys.ap[0], [1, W * S]])
        nc.scalar.activation(
            out=sq, in_=ysf, func=mybir.ActivationFunctionType.Square, accum_out=ss
        )
        nc.sync.dma_start(out=ss_out[:], in_=ss)

        # Stage 4: X0[p, n, l] = sum_s H[p, n, s] * Y[p, n+l, s]
        # s-contraction as an aligned fold tree: 22 -> 16 -> 8 -> 4 -> 2 -> 1
        # VectorE does all multiplies; fold chains are split VectorE/GPSIMD.
        FOLDS = ((0, 16, 6), (0, 8, 8), (0, 4, 4), (0, 2, 2))
        GPS_I4 = set(range(L - gps_indep_s4, L)) if gps_indep_s4 else set()
        # handoff l's: VectorE does the multiply, GPSIMD the fold chain
        GPS_H4 = (
            set(range(L - gps_indep_s4 - gps_hand_s4, L - gps_indep_s4))
            if gps_hand_s4
            else set()
        )
        gpool = ctx.enter_context(tc.tile_pool(name="gpool", bufs=2))
        dve_ls = [l for l in range(L) if l not in GPS_I4 and l not in GPS_H4]

        def ap3(t, pair_step, pairs, d1_step, d1_n, d2_step, d2_n, off):
            return bass.AP(
                tensor=t.tensor,
                offset=t.offset + off,
                ap=[t.ap[0], [pair_step, pairs], [d1_step, d1_n], [d2_step, d2_n]],
            )

        # VectorE side: pair-batched pipelines (one mul + one fold tree per
        # s4_chunk l values, strided across the pair axis of a wide tile).
        i = 0
        while i < len(dve_ls):
            l = dve_ls[i]
            npair = 1
            while (
                npair < s4_chunk
                and i + npair < len(dve_ls)
                and dve_ls[i + npair] == l + npair
            ):
                npair += 1
            i += npair
            tmp = tmps.tile([ROWS, npair, M, S], dtype, tag="tmp")
            nc.vector.tensor_mul(
                out=tmp,
                in0=ap3(hs, 0, npair, S, M, 1, S, 0),
                in1=ap3(ys, S, npair, S, M, 1, S, l * S),
            )
            for dst, src, width in FOLDS:
                o = ap3(tmp, M * S, npair, S, M, 1, width, dst)
                nc.vector.tensor_tensor(
                    out=o,
                    in0=o,
                    in1=ap3(tmp, M * S, npair, S, M, 1, width, src),
                    op=add,
                )
            nc.vector.tensor_tensor(
                out=bass.AP(
                    tensor=x0.tensor,
                    offset=x0.offset + l,
                    ap=[x0.ap[0], [1, npair], [L, M]],
                ),
                in0=ap3(tmp, M * S, npair, S, M, 1, 1, 0)[:, :, :, 0],
                in1=ap3(tmp, M * S, npair, S, M, 1, 1, 1)[:, :, :, 0],
                op=add,
            )
        # GPSIMD side: independent single-l pipelines, full-width (GPSIMD's
        # per-instruction overhead (Q7 launch) makes halving a net loss).
        for l in sorted(GPS_I4 | GPS_H4):
            tmp = gpool.tile([ROWS, M, S], dtype, tag="gtmp")
            mul_eng = nc.vector if l in GPS_H4 else nc.gpsimd
            mul_eng.tensor_mul(out=tmp, in0=hs, in1=ys[:, l : l + M, :])
            for dst, src, width in FOLDS:
                o = tmp[:, :, dst : dst + width]
                nc.gpsimd.tensor_tensor(
                    out=o, in0=o, in1=tmp[:, :, src : src + width], op=add
                )
            nc.gpsimd.tensor_tensor(
                out=x0[:, :, l], in0=tmp[:, :, 0], in1=tmp[:, :, 1], op=add
            )
        nc.sync.dma_start(out=x0_out[:], in_=x0)

    nc.finalize()
    return nc


def shard_inputs(
    x: np.ndarray, H: np.ndarray, np_dtype=np.float16
) -> list[dict[str, np.ndarray]]:
    H2 = H[0, :, :, 0, :]  # (M, M, S)
    x = x.astype(np_dtype)
    H2 = H2.astype(np_dtype)
    ident = np.eye(ROWS, dtype=np_dtype)
    in_maps = []
    for c in range(N_CORES):
        b, half = c // 2, c % 2
        m0 = half * ROWS
        in_maps.append(
            {
                "x_in": np.ascontiguousarray(x[b, m0 : m0 + ROWS]),
                "h_in": np.ascontiguousarray(H2[m0 : m0 + ROWS]),
                "id_in": ident,
            }
        )
    return in_maps


def finalize(
    results: list[dict[str, np.ndarray]],
    H: np.ndarray,
    noise_eps: np.ndarray,
) -> np.ndarray:
    X0 = np.empty((B, M, M, L), np.float32)
    sumsq = 0.0
    for c in range(N_CORES):
        b, half = c // 2, c % 2
        m0 = half * ROWS
        X0[b, m0 : m0 + ROWS] = results[c]["x0_out"].astype(np.float32)
        sumsq += results[c]["ss_out"].sum(dtype=np.float64)
    sigm = sumsq / (M * W * B * 10.0 ** (NOISE_DB / 10.0))

    H2 = H[0, :, :, 0, :]  # (M, M, S)
    hsum = H2.sum(axis=-1)  # (M, M)
    # noise window: nwin[b, m, n, l] = noise_eps[b, m, n + l, 0]
    nwin = np.lib.stride_tricks.sliding_window_view(noise_eps[:, :, :, 0], L, axis=2)
    X = X0 + np.float32(np.sqrt(sigm)) * (hsum[None, :, :, None] * nwin)
    X = X.astype(np.float32, copy=False)
    return X / X.max()


_NC_CACHE: bass.Bass | None = None


def kernel(x: np.ndarray, H: np.ndarray, noise_eps: np.ndarray) -> np.ndarray:
    global _NC_CACHE
    x = np.asarray(x, dtype=np.float32)
    H = np.asarray(H, dtype=np.float32)
    noise_eps = np.asarray(noise_eps, dtype=np.float32)
    if _NC_CACHE is None:
        _NC_CACHE = build_bass()
    in_maps = shard_inputs(x, H)
    res = run_bass_kernel_spmd(_NC_CACHE, in_maps, core_ids=list(range(N_CORES)))
    return finalize(res.results, H, noise_eps)



# revision 29
# speedup vs baseline: 1.1799x; 1.0053x over previous
"""CASSI layer kernel for Trainium2 (8 NeuronCores, Bass/Tile).

Math (matches the reference nn_CASSI_layer):
    H2[m,n,s]        = H[0,m,n,0,s]
    Y[b,m,n+l,s]    += H2[m,n,s] * x[b,m,n,l]            (shear-sum, l in [0,24))
    sigm             = sum(Y^2) / (M*W*B*10^(40/10))
    Yn               = Y + sqrt(sigm) * noise_eps         (noise_eps broadcast over s)
    X[b,m,n,l]       = sum_s H2[m,n,s] * Yn[b,m,n+l,s]
    out              = X / max(X)

Distribution: the (b, m) pairs form 4*256 = 1024 independent rows; each of the
8 cores takes 128 rows (core c: b = c//2, m in [128*(c%2), 128*(c%2)+128)),
mapped onto the 128 SBUF partitions.  Everything per-row lives along the free
dimension, so the spectral shifts are plain address offsets (always 4-byte
aligned in fp16 because the shift stride is S=22 elements).

The two global scalar couplings (sigm, max) are linearized out of the device
kernel: X = X0 + sqrt(sigm)*Xn with X0 the noise-free result (device) and
Xn[b,m,n,l] = (sum_s H2[m,n,s]) * noise_eps[b,m,n+l] (cheap host outer
product).  The device returns X0 and per-partition sum(Y^2); the host applies
sigma, the noise term, and the global max normalization.

Engine split per core: stage 2's shift-ACCUMULATE runs on the TensorEngine
as identity-weight matmuls (lhsT = I_128, a host-shipped constant) that add
each per-l product HX_l = H (.) x_l into PSUM banks (fp32, one 22-wide w
chunk per 2KB bank), with ScalarE flushing finished banks into the fp16 ys
tile; the w axis is swept in 6 passes so only one pass's HX_l slices live in
SBUF.  The HX_l products themselves go to VectorE (fp16 packed 2x off a
ScalarE-materialized broadcast; every 3rd l reads the stride-0 broadcast
directly at 1x to keep ScalarE's copies off the per-pass critical path) and
GPSIMD (last 7 l's, straight off the broadcast AP).  Stage 4 is unchanged:
VectorE/GPSIMD muls + 22->16->8->4->2->1 aligned fold trees, pair-batched;
sum(Y^2) rides the ScalarE Square activation's accumulator (x0 doubles as
the scratch target).  Moving the stage-2 adds to the otherwise-idle PE cut
VectorE's stream from ~226us to ~170us and removed the dual-accumulator
merge; tuned with the calibrated instruction-cost timeline simulator
(250.6us prior all-vector version -> 213.5us; PSUM fp32 accumulation also
halves the relative error vs the fp16 ys accumulator it replaced).  Input
DMAs are split 4-way along n so the first pass's multiplies start ~1/4 of
the way into the (model-serialized) transfer.  Measured dead ends: halving
stage 4 into n-halves for s2/s4 overlap (per-op overhead exceeds the
recovered bubble), full-width GPSIMD stage-2 products (all PSUM flushes
then serialize behind GPSIMD's up-front multiplies), finer GPS/DVE
rebalances, and DVE fold->pool/tensor_reduce swaps (same element count).
"""

from contextlib import ExitStack

import numpy as np

import concourse.bass as bass
import concourse.bacc as bacc
import concourse.tile as tile
from concourse import mybir
from concourse.bass_utils import run_bass_kernel_spmd

B, M, L, S = 4, 256, 24, 22
W = M + L - 1  # 279
N_CORES = 8
ROWS = 128  # (b, m) rows per core
NOISE_DB = 40.0

_F32 = mybir.dt.float32
_F16 = mybir.dt.float16


def build_bass(dtype=_F16, gps_s2=7, gps_hand_s4=0, tmp_bufs=24, rep_bufs=5, gps_indep_s4=5, n_pass=6, psum_bufs=8, s4_chunk=2, wchunk=22, direct_stride=3, dma_split=2) -> bass.Bass:
    nc = bacc.Bacc()
    x_in = nc.declare_dram_parameter("x_in", [ROWS, M, L], dtype, isOutput=False)
    h_in = nc.declare_dram_parameter("h_in", [ROWS, M, S], dtype, isOutput=False)
    id_in = nc.declare_dram_parameter("id_in", [ROWS, ROWS], dtype, isOutput=False)
    x0_out = nc.declare_dram_parameter("x0_out", [ROWS, M, L], dtype, isOutput=True)
    ss_out = nc.declare_dram_parameter("ss_out", [ROWS, 1], _F32, isOutput=True)

    add = mybir.AluOpType.add

    with tile.TileContext(nc) as tc, ExitStack() as ctx:
        main = ctx.enter_context(tc.tile_pool(name="main", bufs=1))
        reps = ctx.enter_context(tc.tile_pool(name="reps", bufs=rep_bufs))
        tmps = ctx.enter_context(tc.tile_pool(name="tmps", bufs=2))
        s2tmps = ctx.enter_context(tc.tile_pool(name="s2tmps", bufs=tmp_bufs))
        ypool = ctx.enter_context(tc.tile_pool(name="ypool", bufs=psum_bufs, space="PSUM"))

        xs = main.tile([ROWS, M, L], dtype, tag="xs")
        hs = main.tile([ROWS, M, S], dtype, tag="hs")
        ident = main.tile([ROWS, ROWS], dtype, tag="ident")
        ys = main.tile([ROWS, W, S], dtype, tag="ys")
        x0 = main.tile([ROWS, M, L], dtype, tag="x0")
        ss = main.tile([ROWS, 1], _F32, tag="ss")

        # Split the input DMAs along n so the first pass's multiplies start
        # after ~1/4 of the transfer instead of all of it (the DMA engines
        # are serialized in the model; splitting only moves the ready-time
        # of the early rows forward).
        step = M // dma_split
        for i in range(dma_split):
            a, b = i * step, (i + 1) * step
            nc.sync.dma_start(out=hs[:, a:b, :], in_=h_in[:, a:b, :])
            nc.sync.dma_start(out=xs[:, a:b, :], in_=x_in[:, a:b, :])
        nc.sync.dma_start(out=ident, in_=id_in[:])

        def x_bcast(l: int, n0: int = 0, n1: int = M) -> bass.AP:
            # x[:, n0:n1, l] broadcast along a trailing s axis: [ROWS, n1-n0, S]
            sl = xs[:, n0:n1, l]
            return bass.AP(
                tensor=sl.tensor, offset=sl.offset, ap=[sl.ap[0], sl.ap[1], [0, S]]
            )

        # Stage 1+2: Y[p, w, s] = sum_l H[p, w-l, s] * x[p, w-l, l]
        # DVE/GPSIMD compute the per-l products HX_l (DVE from a ScalarE-
        # materialized broadcast, packed 2x mode; GPSIMD straight off the
        # broadcast AP); the TensorEngine does the shift-accumulate with
        # identity-weight matmuls into PSUM banks (fp32), and ScalarE
        # flushes each finished bank into the fp16 ys tile.  The w axis is
        # processed in n_pass passes so only a pass's worth of HX_l slices
        # is alive in SBUF at a time.
        GPS_S2 = set(range(L - gps_s2, L)) if gps_s2 else set()
        # w-chunk boundaries: bank-sized pieces of the W axis
        bounds = list(range(0, W, wchunk)) + [W]
        chunks = [(bounds[i], bounds[i + 1]) for i in range(len(bounds) - 1)]
        # group chunks into passes of roughly equal count
        per = (len(chunks) + n_pass - 1) // n_pass
        passes = [chunks[i : i + per] for i in range(0, len(chunks), per)]
        for pchunks in passes:
            P0, P1 = pchunks[0][0], pchunks[-1][1]
            # products for this pass: HX_l[n] for n in [max(0,P0-l), min(M,P1-l))
            ptmp = {}
            for l in range(L):
                nb0, nb1 = max(0, P0 - l), min(M, P1 - l)
                if nb0 >= nb1:
                    continue
                tmp = s2tmps.tile([ROWS, nb1 - nb0, S], dtype, tag="s2tmp")
                ptmp[l] = (tmp, nb0)
                if l in GPS_S2:
                    nc.gpsimd.tensor_mul(out=tmp, in0=hs[:, nb0:nb1, :], in1=x_bcast(l, nb0, nb1))
                elif l == 0 or (direct_stride and l % direct_stride == direct_stride - 1):
                    # direct broadcast read (1x mode): slower on DVE but needs
                    # no ScalarE copy — used to keep ScalarE off the critical
                    # path of each pass
                    nc.vector.tensor_mul(out=tmp, in0=hs[:, nb0:nb1, :], in1=x_bcast(l, nb0, nb1))
                else:
                    xr = reps.tile([ROWS, nb1 - nb0, S], dtype, tag="xr")
                    nc.scalar.copy(out=xr, in_=x_bcast(l, nb0, nb1))
                    nc.vector.tensor_mul(out=tmp, in0=hs[:, nb0:nb1, :], in1=xr)
            for (W0, W1) in pchunks:
                ls = [l for l in range(L) if max(W0, l) < min(W1, l + M)]
                ybank = ypool.tile([ROWS, W1 - W0, S], _F32, tag="ybank")
                for i, l in enumerate(ls):
                    wa, wb = max(W0, l), min(W1, l + M)
                    tmp, nb0 = ptmp[l]
                    nc.tensor.matmul(
                        out=ybank[:, wa - W0 : wb - W0, :],
                        lhsT=ident,
                        rhs=tmp[:, wa - l - nb0 : wb - l - nb0, :],
                        start=(i == 0),
                        stop=(i == len(ls) - 1),
                        skip_group_check=True,
                    )
                nc.scalar.copy(out=ys[:, W0:W1, :], in_=ybank)

        # Stage 3 partial: per-partition sum(Y^2) via ScalarE Square+accumulate.
        # x0 is not written until stage 4's pipelines finish their folds, so
        # it doubles as the Square target (flat [W*S] slice of its [M*L]).
        sq = bass.AP(tensor=x0.tensor, offset=x0.offset, ap=[x0.ap[0], [1, W * S]])
        ysf = bass.AP(tensor=ys.tensor, offset=ys.offset, ap=[ys.ap[0], [1, W * S]])
        nc.scalar.activation(
            out=sq, in_=ysf, func=mybir.ActivationFunctionType.Square, accum_out=ss
        )
        nc.sync.dma_start(out=ss_out[:], in_=ss)

        # Stage 4: X0[p, n, l] = sum_s H[p, n, s] * Y[p, n+l, s]
        # s-contraction as an aligned fold tree: 22 -> 16 -> 8 -> 4 -> 2 -> 1
        # VectorE does all multiplies; fold chains are split VectorE/GPSIMD.
        FOLDS = ((0, 16, 6), (0, 8, 8), (0, 4, 4), (0, 2, 2))
        GPS_I4 = set(range(L - gps_indep_s4, L)) if gps_indep_s4 else set()
        # handoff l's: VectorE does the multiply, GPSIMD the fold chain
        GPS_H4 = (
            set(range(L - gps_indep_s4 - gps_hand_s4, L - gps_indep_s4))
            if gps_hand_s4
            else set()
        )
        gpool = ctx.enter_context(tc.tile_pool(name="gpool", bufs=2))
        dve_ls = [l for l in range(L) if l not in GPS_I4 and l not in GPS_H4]

        def ap3(t, pair_step, pairs, d1_step, d1_n, d2_step, d2_n, off):
            return bass.AP(
                tensor=t.tensor,
                offset=t.offset + off,
                ap=[t.ap[0], [pair_step, pairs], [d1_step, d1_n], [d2_step, d2_n]],
            )

        # VectorE side: pair-batched pipelines (one mul + one fold tree per
        # s4_chunk l values, strided across the pair axis of a wide tile).
        i = 0
        while i < len(dve_ls):
            l = dve_ls[i]
            npair = 1
            while (
                npair < s4_chunk
                and i + npair < len(dve_ls)
                and dve_ls[i + npair] == l + npair
            ):
                npair += 1
            i += npair
            tmp = tmps.tile([ROWS, npair, M, S], dtype, tag="tmp")
            nc.vector.tensor_mul(
                out=tmp,
                in0=ap3(hs, 0, npair, S, M, 1, S, 0),
                in1=ap3(ys, S, npair, S, M, 1, S, l * S),
            )
            for dst, src, width in FOLDS:
                o = ap3(tmp, M * S, npair, S, M, 1, width, dst)
                nc.vector.tensor_tensor(
                    out=o,
                    in0=o,
                    in1=ap3(tmp, M * S, npair, S, M, 1, width, src),
                    op=add,
                )
            nc.vector.tensor_tensor(
                out=bass.AP(
                    tensor=x0.tensor,
                    offset=x0.offset + l,
                    ap=[x0.ap[0], [1, npair], [L, M]],
                ),
                in0=ap3(tmp, M * S, npair, S, M, 1, 1, 0)[:, :, :, 0],
                in1=ap3(tmp, M * S, npair, S, M, 1, 1, 1)[:, :, :, 0],
                op=add,
            )
        # GPSIMD side: independent single-l pipelines, full-width (GPSIMD's
        # per-instruction overhead (Q7 launch) makes halving a net loss).
        for l in sorted(GPS_I4 | GPS_H4):
            tmp = gpool.tile([ROWS, M, S], dtype, tag="gtmp")
            mul_eng = nc.vector if l in GPS_H4 else nc.gpsimd
            mul_eng.tensor_mul(out=tmp, in0=hs, in1=ys[:, l : l + M, :])
            for dst, src, width in FOLDS:
                o = tmp[:, :, dst : dst + width]
                nc.gpsimd.tensor_tensor(
                    out=o, in0=o, in1=tmp[:, :, src : src + width], op=add
                )
            nc.gpsimd.tensor_tensor(
                out=x0[:, :, l], in0=tmp[:, :, 0], in1=tmp[:, :, 1], op=add
            )
        nc.sync.dma_start(out=x0_out[:], in_=x0)

    nc.finalize()
    return nc


def shard_inputs(
    x: np.ndarray, H: np.ndarray, np_dtype=np.float16
) -> list[dict[str, np.ndarray]]:
    H2 = H[0, :, :, 0, :]  # (M, M, S)
    x = x.astype(np_dtype)
    H2 = H2.astype(np_dtype)
    ident = np.eye(ROWS, dtype=np_dtype)
    in_maps = []
    for c in range(N_CORES):
        b, half = c // 2, c % 2
        m0 = half * ROWS
        in_maps.append(
            {
                "x_in": np.ascontiguousarray(x[b, m0 : m0 + ROWS]),
                "h_in": np.ascontiguousarray(H2[m0 : m0 + ROWS]),
                "id_in": ident,
            }
        )
    return in_maps


def finalize(
    results: list[dict[str, np.ndarray]],
    H: np.ndarray,
    noise_eps: np.ndarray,
) -> np.ndarray:
    X0 = np.empty((B, M, M, L), np.float32)
    sumsq = 0.0
    for c in range(N_CORES):
        b, half = c // 2, c % 2
        m0 = half * ROWS
        X0[b, m0 : m0 + ROWS] = results[c]["x0_out"].astype(np.float32)
        sumsq += results[c]["ss_out"].sum(dtype=np.float64)
    sigm = sumsq / (M * W * B * 10.0 ** (NOISE_DB / 10.0))

    H2 = H[0, :, :, 0, :]  # (M, M, S)
    hsum = H2.sum(axis=-1)  # (M, M)
    # noise window: nwin[b, m, n, l] = noise_eps[b, m, n + l, 0]
    nwin = np.lib.stride_tricks.sliding_window_view(noise_eps[:, :, :, 0], L, axis=2)
    X = X0 + np.float32(np.sqrt(sigm)) * (hsum[None, :, :, None] * nwin)
    X = X.astype(np.float32, copy=False)
    return X / X.max()


_NC_CACHE: bass.Bass | None = None


def kernel(x: np.ndarray, H: np.ndarray, noise_eps: np.ndarray) -> np.ndarray:
    global _NC_CACHE
    x = np.asarray(x, dtype=np.float32)
    H = np.asarray(H, dtype=np.float32)
    noise_eps = np.asarray(noise_eps, dtype=np.float32)
    if _NC_CACHE is None:
        _NC_CACHE = build_bass()
    in_maps = shard_inputs(x, H)
    res = run_bass_kernel_spmd(_NC_CACHE, in_maps, core_ids=list(range(N_CORES)))
    return finalize(res.results, H, noise_eps)

